# revision 7
# baseline (speedup 1.0000x reference)
"""BasisVQ Trainium2 kernel.

reference(latent_coeffs, basis_vectors):
    probs = softmax(latent * 30, -1); idx = argmax(probs, -1)
    one_hot_st = probs + stop_gradient(one_hot(idx) - probs)   # value == one_hot exactly in fp32
    quantized = one_hot_st @ basis                             # == basis[idx]
    return quantized, idx

Softmax is monotonic, and (0 - p) + p == 0 / (1 - p) + p == 1 exactly in fp32,
so the forward value is exactly (basis[argmax(latent, -1)], argmax(latent, -1)).

Kernel: data-parallel over 8 cores, 4096 tokens per core.
Per core: DVE max/max_index for the argmax over C=1024, then an indirect DMA
row-gather of the [1024, 900] basis table (kept in HBM) into SBUF, streamed
back out to HBM.

Core-local layout: partition p owns tokens p*G..p*G+G-1 (G=32), so all DRAM
tensors are declared in the [128, G*width] layout, which is just a reshape of
the contiguous token shard on the host.
"""

import numpy as np

import concourse.bacc as bacc
import concourse.bass as bass
import concourse.mybir as mybir
from concourse.bass_utils import run_bass_kernel_spmd
from concourse.tile import TileContext

N_CORES = 8
B, K, C, D = 16, 2048, 1024, 900
TOK = B * K                      # 32768
TPC = TOK // N_CORES             # 4096 tokens per core
P = 128
G = TPC // P                     # 32 token-groups per core
CHUNK_G = 4                      # groups per latent load chunk (2 MiB DMAs)
N_CHUNKS = G // CHUNK_G

_CACHE = {}


def _build():
    nc = bacc.Bacc(None, target_bir_lowering=False)
    latent = nc.dram_tensor(
        "latent", [P, G * C], mybir.dt.float32, kind="ExternalInput"
    )
    basis = nc.dram_tensor(
        "basis", [1024, D], mybir.dt.float32, kind="ExternalInput"
    )
    quant = nc.dram_tensor(
        "quantized", [P, G * D], mybir.dt.float32, kind="ExternalOutput"
    )
    indices = nc.dram_tensor(
        "indices", [P, G], mybir.dt.int32, kind="ExternalOutput"
    )

    with TileContext(nc) as tc:
        with (
            tc.tile_pool(name="lat", bufs=2) as lat_pool,
            tc.tile_pool(name="gath", bufs=G) as gath_pool,
            tc.tile_pool(name="small", bufs=G) as small_pool,
            tc.tile_pool(name="persist", bufs=1) as persist_pool,
        ):
            idx_acc = persist_pool.tile([P, G], mybir.dt.int32)
            for ch in range(N_CHUNKS):
                lat_tile = lat_pool.tile([P, CHUNK_G * C], mybir.dt.float32, tag="lat")
                nc.sync.dma_start(
                    out=lat_tile[:],
                    in_=latent[:, ch * CHUNK_G * C : (ch + 1) * CHUNK_G * C],
                )
                for gl in range(CHUNK_G):
                    g = ch * CHUNK_G + gl
                    vals = lat_tile[:, gl * C : (gl + 1) * C]
                    max8 = small_pool.tile([P, 8], mybir.dt.float32, tag="max8")
                    idx8 = small_pool.tile([P, 8], mybir.dt.uint32, tag="idx8")
                    nc.vector.max(max8[:], vals)
                    nc.vector.max_index(idx8[:], max8[:], vals)
                    nc.vector.tensor_copy(
                        out=idx_acc[:, g : g + 1], in_=idx8[:, 0:1]
                    )
                    gath = gath_pool.tile([P, D], mybir.dt.float32, tag="gath")
                    nc.gpsimd.indirect_dma_start(
                        out=gath[:],
                        out_offset=None,
                        in_=basis[:],
                        in_offset=bass.IndirectOffsetOnAxis(ap=idx8[:, 0:1], axis=0),
                    )
                    nc.sync.dma_start(
                        out=quant[:, g * D : (g + 1) * D], in_=gath[:]
                    )
            nc.sync.dma_start(out=indices[:], in_=idx_acc[:])
    nc.compile()
    return nc


def kernel(latent_coeffs: np.ndarray, basis_vectors: np.ndarray):
    if "nc" not in _CACHE:
        _CACHE["nc"] = _build()
    nc = _CACHE["nc"]

    lat = np.ascontiguousarray(latent_coeffs, dtype=np.float32).reshape(TOK, C)
    basis = np.ascontiguousarray(basis_vectors, dtype=np.float32)
    in_maps = [
        {
            "latent": lat[c * TPC : (c + 1) * TPC].reshape(P, G * C),
            "basis": basis,
        }
        for c in range(N_CORES)
    ]
    res = run_bass_kernel_spmd(nc, in_maps, list(range(N_CORES)))
    quant = np.concatenate(
        [res.results[c]["quantized"].reshape(TPC, D) for c in range(N_CORES)]
    ).reshape(B, K, D)
    idx = np.concatenate(
        [res.results[c]["indices"].reshape(TPC) for c in range(N_CORES)]
    ).reshape(B, K)
    return quant, idx.astype(np.int32)


# revision 9
# speedup vs baseline: 1.2352x; 1.2352x over previous
"""BasisVQ Trainium2 kernel.

reference(latent_coeffs, basis_vectors):
    probs = softmax(latent * 30, -1); idx = argmax(probs, -1)
    one_hot_st = probs + stop_gradient(one_hot(idx) - probs)   # value == one_hot exactly in fp32
    quantized = one_hot_st @ basis                             # == basis[idx]
    return quantized, idx

Softmax is monotonic, and (0 - p) + p == 0 / (1 - p) + p == 1 exactly in fp32,
so the forward value is exactly (basis[argmax(latent, -1)], argmax(latent, -1)).

Kernel: data-parallel over 8 cores, 4096 tokens per core.
Per core: DVE max/max_index for the argmax over C=1024, then an indirect DMA
row-gather of the [1024, 900] basis table (kept in HBM) into SBUF, streamed
back out to HBM.

Core-local layout: partition p owns tokens p*G..p*G+G-1 (G=32), so all DRAM
tensors are declared in the [128, G*width] layout, which is just a reshape of
the contiguous token shard on the host.
"""

import numpy as np

import concourse.bacc as bacc
import concourse.bass as bass
import concourse.mybir as mybir
from concourse.bass_utils import run_bass_kernel_spmd
from concourse.tile import TileContext

N_CORES = 8
B, K, C, D = 16, 2048, 1024, 900
TOK = B * K                      # 32768
TPC = TOK // N_CORES             # 4096 tokens per core
P = 128
G = TPC // P                     # 32 token-groups per core
CHUNK_G = 4                      # groups per latent load chunk (2 MiB DMAs)
N_CHUNKS = G // CHUNK_G

_CACHE = {}


def _build():
    nc = bacc.Bacc(None, target_bir_lowering=False)
    latent = nc.dram_tensor(
        "latent", [P, G * C], mybir.dt.float32, kind="ExternalInput"
    )
    basis = nc.dram_tensor(
        "basis", [1024, D], mybir.dt.float32, kind="ExternalInput"
    )
    quant = nc.dram_tensor(
        "quantized", [P, G * D], mybir.dt.float32, kind="ExternalOutput"
    )
    indices = nc.dram_tensor(
        "indices", [P, G], mybir.dt.int32, kind="ExternalOutput"
    )

    with TileContext(nc) as tc:
        with (
            tc.tile_pool(name="lat", bufs=3) as lat_pool,
            tc.tile_pool(name="gath", bufs=G) as gath_pool,
            tc.tile_pool(name="small", bufs=G) as small_pool,
            tc.tile_pool(name="persist", bufs=1) as persist_pool,
        ):
            idx_acc = persist_pool.tile([P, G], mybir.dt.int32)
            for ch in range(N_CHUNKS):
                lat_tile = lat_pool.tile([P, CHUNK_G * C], mybir.dt.float32, tag="lat")
                nc.sync.dma_start(
                    out=lat_tile[:],
                    in_=latent[:, ch * CHUNK_G * C : (ch + 1) * CHUNK_G * C],
                )
                for gl in range(CHUNK_G):
                    g = ch * CHUNK_G + gl
                    vals = lat_tile[:, gl * C : (gl + 1) * C]
                    max8 = small_pool.tile([P, 8], mybir.dt.float32, tag="max8")
                    idx8 = small_pool.tile([P, 8], mybir.dt.uint32, tag="idx8")
                    nc.vector.max(max8[:], vals)
                    nc.vector.max_index(idx8[:], max8[:], vals)
                    nc.vector.tensor_copy(
                        out=idx_acc[:, g : g + 1], in_=idx8[:, 0:1]
                    )
                    gath = gath_pool.tile([P, D], mybir.dt.float32, tag="gath")
                    nc.gpsimd.indirect_dma_start(
                        out=gath[:],
                        out_offset=None,
                        in_=basis[:],
                        in_offset=bass.IndirectOffsetOnAxis(ap=idx8[:, 0:1], axis=0),
                    )
                    nc.scalar.dma_start(
                        out=quant[:, g * D : (g + 1) * D], in_=gath[:]
                    )
            nc.scalar.dma_start(out=indices[:], in_=idx_acc[:])
    nc.compile()
    return nc


def kernel(latent_coeffs: np.ndarray, basis_vectors: np.ndarray):
    if "nc" not in _CACHE:
        _CACHE["nc"] = _build()
    nc = _CACHE["nc"]

    lat = np.ascontiguousarray(latent_coeffs, dtype=np.float32).reshape(TOK, C)
    basis = np.ascontiguousarray(basis_vectors, dtype=np.float32)
    in_maps = [
        {
            "latent": lat[c * TPC : (c + 1) * TPC].reshape(P, G * C),
            "basis": basis,
        }
        for c in range(N_CORES)
    ]
    res = run_bass_kernel_spmd(nc, in_maps, list(range(N_CORES)))
    quant = np.concatenate(
        [res.results[c]["quantized"].reshape(TPC, D) for c in range(N_CORES)]
    ).reshape(B, K, D)
    idx = np.concatenate(
        [res.results[c]["indices"].reshape(TPC) for c in range(N_CORES)]
    ).reshape(B, K)
    return quant, idx.astype(np.int32)
